# revision 1
# baseline (speedup 1.0000x reference)
"""Adaptive avg pool 2D (16,768,64,48) -> (16,768,7,7) on 8 TRN2 NeuronCores.

Data-parallel over B*C rows: 12288 rows of 64*48=3072 f32, 1536 rows/core.
Per 128-row tile: H-pool (one strided reduce_sum; windows all size 10,
stride 9) then W-pool (two grouped reduce_sums; sizes 7/8), then a
per-element scale. Stores ride gpsimd so their DVE waits never stall the
SP load pipeline; a post-Tile pass legalizes multi-wait sync for this
walrus (max 1 wait/instruction, 2 on EventSemaphore).
  W windows (48->7): q=0:[0,7) q=6:[41,48) size 7; q=1..5 start 6+7(q-1) size 8
  H windows (64->7): start 9*o, size 10 for all o
"""

import sys

_TRN_REPO = "/opt/trn_rl_repo"
if _TRN_REPO not in sys.path:
    sys.path.insert(0, _TRN_REPO)

import numpy as np

import concourse.bass as bass
import concourse.mybir as mybir
from concourse.tile import TileContext

B, C, H, W = 16, 768, 64, 48
HO, WO = 7, 7
NCORES = 8
ROWS = B * C // NCORES  # 1536 rows per core
P = 128
NTILES = ROWS // P  # 12
SPLIT_LAST = 2  # how many trailing tiles use the two-chunk load

_nc_cache = None


def _legalize_multiwait(nc: bass.Bass) -> None:
    """Walrus (this version) accepts at most one sync wait per instruction
    (two for EventSemaphore). Tile's sem assignment can emit more (e.g. the
    kernel-tail drain waits on every DMA queue sem). Hoist all but the last
    wait into dedicated single-wait EventSemaphore carriers placed directly
    before the offending instruction on the same engine."""
    n = 0
    for b in nc.m.functions[0].blocks:
        insts = b.instructions
        i = 0
        while i < len(insts):
            inst = insts[i]
            si = inst.sync_info
            if si is not None and len(si.on_wait) > 1:
                waits = list(si.on_wait)
                carriers = []
                rest = waits[:-1]
                # EventSemaphore carriers can hold 2 waits each.
                for j in range(0, len(rest), 2):
                    n += 1
                    ev = mybir.InstEventSemaphore(
                        name=f"I-waitfix-{n}", ins=[], outs=[]
                    )
                    ev.engine = inst.engine
                    ev.sync_info = mybir.SyncInfo(
                        on_wait=rest[j : j + 2], on_update=[]
                    )
                    nc.register_instruction(ev)
                    carriers.append(ev)
                inst.sync_info = mybir.SyncInfo(
                    on_wait=[waits[-1]], on_update=list(si.on_update)
                )
                insts[i:i] = carriers
                i += len(carriers)
            i += 1


def _build() -> bass.Bass:
    nc = bass.Bass()
    x = nc.dram_tensor("x", [ROWS, H * W], mybir.dt.float32, kind="ExternalInput")
    out = nc.dram_tensor(
        "out", [ROWS, HO * WO], mybir.dt.float32, kind="ExternalOutput"
    )
    f32 = mybir.dt.float32
    X = mybir.AxisListType.X
    with TileContext(nc) as tc:
        with (
            tc.tile_pool(name="xp", bufs=NTILES) as xp,
            tc.tile_pool(name="tp", bufs=3) as tp,
            tc.tile_pool(name="op", bufs=3) as op,
            tc.tile_pool(name="sp", bufs=NTILES) as sp,
            tc.tile_pool(name="cp", bufs=1) as cp,
        ):
            # Scale tile: sc[p, o*7+q] = 1/(10 * wsize_q); wsize = 7 for
            # q in {0,6}, 8 for q in 1..5. Same for every o.
            sc = cp.tile([P, HO * WO], f32)
            ps = list(sc.ap[0])
            nc.vector.memset(
                bass.AP(tensor=sc.tensor, offset=sc.offset, ap=[ps, [WO, HO], [6, 2]]),
                1.0 / 70.0,
            )
            nc.vector.memset(
                bass.AP(
                    tensor=sc.tensor, offset=sc.offset + 1, ap=[ps, [WO, HO], [1, 5]]
                ),
                1.0 / 80.0,
            )
            # Tiles >= NTILES - SPLIT_LAST load in two column chunks at the
            # h=27 window boundary (o 0-2 need h<28, o 3-6 need h>=27), so
            # the H-reduce of chunk A overlaps chunk B's transfer. This lets
            # DVE converge from load+5.1us to load+3.7us over the last few
            # tiles, shrinking the post-stream tail.
            HA = 28  # rows 0..27 cover o=0..2
            HB = 27  # rows 27..63 cover o=3..6
            for i in range(NTILES):
                rows = x[i * P : (i + 1) * P, :].rearrange(
                    "p (h w) -> p h w", w=W
                )
                tH = tp.tile([P, HO, W], f32)
                ph = list(tH.ap[0])
                if i >= NTILES - SPLIT_LAST:
                    xa = xp.tile([P, HA, W], f32, tag="xa", bufs=2)
                    xb = xp.tile([P, H - HB, W], f32, tag="xb", bufs=2)
                    nc.sync.dma_start(out=xa, in_=rows[:, :HA, :])
                    nc.sync.dma_start(out=xb, in_=rows[:, HB:, :])
                    nc.vector.reduce_sum(
                        out=tH[:, 0:3, :],
                        in_=bass.AP(
                            tensor=xa.tensor,
                            offset=xa.offset,
                            ap=[list(xa.ap[0]), [9 * W, 3], [1, W], [W, 10]],
                        ),
                        axis=X,
                    )
                    nc.vector.reduce_sum(
                        out=tH[:, 3:7, :],
                        in_=bass.AP(
                            tensor=xb.tensor,
                            offset=xb.offset,
                            ap=[list(xb.ap[0]), [9 * W, 4], [1, W], [W, 10]],
                        ),
                        axis=X,
                    )
                else:
                    xt = xp.tile([P, H, W], f32)
                    nc.sync.dma_start(out=xt, in_=rows)
                    pt = list(xt.ap[0])
                    # H pool (all windows size 10, stride 9) in one reduce:
                    # tH[p, o, w] = sum_{h in [9o, 9o+10)} x[p, h, w]
                    nc.vector.reduce_sum(
                        out=tH,
                        in_=bass.AP(
                            tensor=xt.tensor,
                            offset=xt.offset,
                            ap=[pt, [9 * W, HO], [1, W], [W, 10]],
                        ),
                        axis=X,
                    )
                # W pool on tH: q in {0, 6} (size-7 windows at w = 0 and 41)
                ot = op.tile([P, HO, WO], f32)
                po = list(ot.ap[0])
                nc.vector.reduce_sum(
                    out=bass.AP(
                        tensor=ot.tensor,
                        offset=ot.offset,
                        ap=[po, [WO, HO], [6, 2]],
                    ),
                    in_=bass.AP(
                        tensor=tH.tensor,
                        offset=tH.offset,
                        ap=[ph, [W, HO], [41, 2], [1, 7]],
                    ),
                    axis=X,
                )
                # q in 1..5: size-8 windows starting at 6 + 7*(q-1)
                nc.vector.reduce_sum(
                    out=bass.AP(
                        tensor=ot.tensor,
                        offset=ot.offset + 1,
                        ap=[po, [WO, HO], [1, 5]],
                    ),
                    in_=bass.AP(
                        tensor=tH.tensor,
                        offset=tH.offset + 6,
                        ap=[ph, [W, HO], [7, 5], [1, 8]],
                    ),
                    axis=X,
                )
                os_ = sp.tile([P, HO * WO], f32)
                last = i == NTILES - 1
                if last:
                    # Tail path: DVE and SP are both idle by now. DVE mul
                    # avoids the cross-engine hop; SP store descgen (~0.6us)
                    # beats Pool SWDGE (~1.0us), and its DVE wait can no
                    # longer block loads (all issued).
                    nc.vector.tensor_mul(
                        os_, ot.rearrange("p a b -> p (a b)"), sc
                    )
                    nc.sync.dma_start(
                        out=out[i * P : (i + 1) * P, :],
                        in_=os_,
                    )
                else:
                    # Steady state: scale on gpsimd keeps DVE under the DMA
                    # period; store on gpsimd so its DVE wait never blocks
                    # SP load issue.
                    nc.gpsimd.tensor_mul(
                        os_, ot.rearrange("p a b -> p (a b)"), sc
                    )
                    nc.gpsimd.dma_start(
                        out=out[i * P : (i + 1) * P, :],
                        in_=os_,
                    )
    _legalize_multiwait(nc)
    return nc


def kernel(x: np.ndarray) -> np.ndarray:
    global _nc_cache
    from concourse.bass_utils import run_bass_kernel_spmd

    xr = np.ascontiguousarray(np.asarray(x, dtype=np.float32).reshape(B * C, H * W))
    if _nc_cache is None:
        _nc_cache = _build()
    nc = _nc_cache
    in_maps = [
        {"x": xr[k * ROWS : (k + 1) * ROWS]} for k in range(NCORES)
    ]
    res = run_bass_kernel_spmd(nc, in_maps, list(range(NCORES)))
    out = np.concatenate([r["out"] for r in res.results], axis=0)
    return out.reshape(B, C, HO, WO)



# revision 2
# speedup vs baseline: 1.6202x; 1.6202x over previous
"""Adaptive avg pool 2D (16,768,64,48) -> (16,768,7,7) on 8 TRN2 NeuronCores.

Data-parallel over B*C rows: 12288 rows of 64*48=3072 f32, 1536 rows/core.

Key idea vs the f32 baseline: the DMA cost is proportional to the SBUF-side
bytes, and gpsimd (SWDGE) DMAs can cast. Loading f32 DRAM -> f16 SBUF halves
the per-tile transfer (4369ns -> 2184ns), dropping the per-core DMA floor
from ~52.4us to ~26.2us. Input magnitudes are N(0,1) so f16 keeps rel err
~3e-4, far under the 2e-2 gate.

Per 128-row tile (all [p, h, w] with w innermost, f16):
  H pool (windows size 10, stride 9) as a 4-instruction pairwise add tree
  on DVE: tensor_tensor adds run in 2x mode for packed 16-bit operands
  (0.52 ns/elem) while TensorReduce is always 1 elem/cycle. 10 rows ->
  5 pair sums -> 2 -> 1 (+ leftover pair): 3024 elem-adds ~= 1.8us.
  W pool (windows 7,8,8,8,8,8,7) as two small strided reduce_sums reading
  7x54 f16 elems (~0.5us, 1x).
  Scale on the Activation engine as two immediate-scale Copy activations
  (1/70 for q in {0,6}, 1/80 else), f16 -> f32, into a [128, 588] staging
  buffer; stores ride Act's HWDGE so neither DVE nor the Pool SEQ (busy
  with SWDGE descriptor gen for the cast loads) ever stalls.

Output DRAM layout is [128, 12*49] (tile-major columns); the host reorders
to [1536, 49]. A post-Tile pass legalizes multi-wait sync for this walrus
(max 1 wait/instruction, 2 on EventSemaphore).
  W windows (48->7): q=0:[0,7) q=6:[41,48) size 7; q=1..5 start 6+7(q-1) size 8
  H windows (64->7): start 9*o, size 10 for all o
"""

import sys

_TRN_REPO = "/opt/trn_rl_repo"
if _TRN_REPO not in sys.path:
    sys.path.insert(0, _TRN_REPO)

import numpy as np

import concourse.bass as bass
import concourse.mybir as mybir
from concourse.tile import TileContext

B, C, H, W = 16, 768, 64, 48
HO, WO = 7, 7
NCORES = 8
ROWS = B * C // NCORES  # 1536 rows per core
P = 128
NTILES = ROWS // P  # 12

_nc_cache = None


def _legalize_multiwait(nc: bass.Bass) -> None:
    """Walrus (this version) accepts at most one sync wait per instruction
    (two for EventSemaphore). Tile's sem assignment can emit more (e.g. the
    kernel-tail drain waits on every DMA queue sem). Hoist all but the last
    wait into dedicated single-wait EventSemaphore carriers placed directly
    before the offending instruction on the same engine."""
    n = 0
    for b in nc.m.functions[0].blocks:
        insts = b.instructions
        i = 0
        while i < len(insts):
            inst = insts[i]
            si = inst.sync_info
            if si is not None and len(si.on_wait) > 1:
                waits = list(si.on_wait)
                carriers = []
                rest = waits[:-1]
                # EventSemaphore carriers can hold 2 waits each.
                for j in range(0, len(rest), 2):
                    n += 1
                    ev = mybir.InstEventSemaphore(
                        name=f"I-waitfix-{n}", ins=[], outs=[]
                    )
                    ev.engine = inst.engine
                    ev.sync_info = mybir.SyncInfo(
                        on_wait=rest[j : j + 2], on_update=[]
                    )
                    nc.register_instruction(ev)
                    carriers.append(ev)
                inst.sync_info = mybir.SyncInfo(
                    on_wait=[waits[-1]], on_update=list(si.on_update)
                )
                insts[i:i] = carriers
                i += len(carriers)
            i += 1


def _build() -> bass.Bass:
    nc = bass.Bass()
    x = nc.dram_tensor("x", [ROWS, H * W], mybir.dt.float32, kind="ExternalInput")
    out = nc.dram_tensor(
        "out", [P, NTILES * HO * WO], mybir.dt.float32, kind="ExternalOutput"
    )
    f16 = mybir.dt.float16
    f32 = mybir.dt.float32
    X = mybir.AxisListType.X
    Copy = mybir.ActivationFunctionType.Copy

    def ap(tile, off, dims):
        return bass.AP(
            tensor=tile.tensor, offset=tile.offset + off,
            ap=[list(tile.ap[0])] + dims,
        )

    with TileContext(nc) as tc:
        with (
            tc.tile_pool(name="xp", bufs=3) as xp,
            tc.tile_pool(name="yp", bufs=2) as yp,
            tc.tile_pool(name="hp", bufs=2) as hp,
            tc.tile_pool(name="op", bufs=2) as op,
            tc.tile_pool(name="sp", bufs=1) as sp,
        ):
            os_ = sp.tile([P, NTILES * HO * WO], f32)
            for i in range(NTILES):
                rows = x[i * P : (i + 1) * P, :]
                xt = xp.tile([P, H * W], f16)
                nc.gpsimd.dma_start(out=xt, in_=rows)
                # H pool: tH[o, w] = sum_{j<10} xt[9o+j, w]; pairwise tree.
                y1 = yp.tile([P, HO * 5 * W], f16, tag="y1")  # [o, k, w] k<5
                y2 = yp.tile([P, HO * 2 * W], f16, tag="y2")  # [o, k2, w]
                tH = hp.tile([P, HO * W], f16)  # [o, w]
                nc.vector.tensor_add(
                    ap(y1, 0, [[5 * W, HO], [W, 5], [1, W]]),
                    ap(xt, 0, [[9 * W, HO], [2 * W, 5], [1, W]]),
                    ap(xt, W, [[9 * W, HO], [2 * W, 5], [1, W]]),
                )
                nc.vector.tensor_add(
                    ap(y2, 0, [[2 * W, HO], [W, 2], [1, W]]),
                    ap(y1, 0, [[5 * W, HO], [2 * W, 2], [1, W]]),
                    ap(y1, W, [[5 * W, HO], [2 * W, 2], [1, W]]),
                )
                nc.vector.tensor_add(
                    ap(tH, 0, [[W, HO], [1, W]]),
                    ap(y2, 0, [[2 * W, HO], [1, W]]),
                    ap(y2, W, [[2 * W, HO], [1, W]]),
                )
                nc.vector.tensor_add(
                    ap(tH, 0, [[W, HO], [1, W]]),
                    ap(tH, 0, [[W, HO], [1, W]]),
                    ap(y1, 4 * W, [[5 * W, HO], [1, W]]),
                )
                # W pool on tH -> ot[o, q] (f16; reduces are 1x anyway but
                # small). q in {0,6}: size-7 windows at w = 0 and 41.
                ot = op.tile([P, HO * WO], f16)
                with nc.allow_low_precision(reason="f16 sums, |x|~N(0,1)"):
                    nc.vector.reduce_sum(
                        out=ap(ot, 0, [[WO, HO], [6, 2]]),
                        in_=ap(tH, 0, [[W, HO], [41, 2], [1, 7]]),
                        axis=X,
                    )
                    nc.vector.reduce_sum(
                        out=ap(ot, 1, [[WO, HO], [1, 5]]),
                        in_=ap(tH, 6, [[W, HO], [7, 5], [1, 8]]),
                        axis=X,
                    )
                # Scale on Act engine: out = in * 1/(10*wsize_q), f16 -> f32.
                col = i * HO * WO
                nc.scalar.activation(
                    ap(os_, col, [[WO, HO], [6, 2]]),
                    ap(ot, 0, [[WO, HO], [6, 2]]),
                    Copy, scale=1.0 / 70.0,
                )
                nc.scalar.activation(
                    ap(os_, col + 1, [[WO, HO], [1, 5]]),
                    ap(ot, 1, [[WO, HO], [1, 5]]),
                    Copy, scale=1.0 / 80.0,
                )
                # Store this tile's 49 columns from the staging buffer.
                nc.scalar.dma_start(
                    out=out[:, col : col + HO * WO],
                    in_=ap(os_, col, [[1, HO * WO]]),
                )
    _legalize_multiwait(nc)
    return nc


def kernel(x: np.ndarray) -> np.ndarray:
    global _nc_cache
    from concourse.bass_utils import run_bass_kernel_spmd

    xr = np.ascontiguousarray(np.asarray(x, dtype=np.float32).reshape(B * C, H * W))
    if _nc_cache is None:
        _nc_cache = _build()
    nc = _nc_cache
    in_maps = [
        {"x": xr[k * ROWS : (k + 1) * ROWS]} for k in range(NCORES)
    ]
    res = run_bass_kernel_spmd(nc, in_maps, list(range(NCORES)))
    # Per-core out is [128, NTILES*49] tile-major; reorder to [1536, 49].
    parts = [
        r["out"].reshape(P, NTILES, HO * WO).transpose(1, 0, 2).reshape(ROWS, HO * WO)
        for r in res.results
    ]
    return np.concatenate(parts, axis=0).reshape(B, C, HO, WO)
